# revision 15
# baseline (speedup 1.0000x reference)
"""Trainium2 Bass kernel: 16-head causal attention with RoPE (seq=4096, feat=1024).

Sharding: tensor-parallel on heads - 2 heads per core across 8 NeuronCores.
Each core computes the qkv projection for its 2 heads, RoPE, causal softmax
attention, and writes a (2*65, 4096) output slab (head outputs transposed,
plus fused softmax denominators); the host divides/transposes/concatenates.

Fused-pipeline design (v2):
  - Single loop over 512-wide seq chunks. Chunk c's projection matmuls
    (stage A) are spread between chunk c-1's attention groups (stage B) so
    the scalar engine - saturated by exp, the critical path - never idles
    across phase boundaries, and the PE stays busy enough to hold its fast
    p-state (it drops to 1.2 GHz after multi-us idle gaps).
  - Scores transposed ST = K @ QT with both heads concurrent via PE row
    tiling (K=64 at partition bases 0/64); one exp per (kt, h0+h1) pair via
    a 2D-free access pattern; P = exp(ST/8) on the scalar engine.
  - Diagonal tiles narrowed: only q >= 128*m is computed/exp'd/accumulated,
    and the residual mask zone is a single m-invariant [128,128] tril
    (pass iff p <= j), multiplied post-exp on the DVE in bf16.
  - V reaches natural (s, d) layout via PE transpose (bf16 PSUM out) - no
    DRAM round-trip - then lands in per-chunk vnat tiles whose ones columns
    accumulate softmax denominators for free during PV.
  - PV accumulates oT[65, 512] per head in PSUM across all kt of a q-chunk.
"""

import sys

if "/opt/trn_rl_repo" not in sys.path:
    sys.path.insert(0, "/opt/trn_rl_repo")

import numpy as np
import ml_dtypes

S = 4096
F = 1024
NH = 16
HD = 64
NCORES = 8
CH = 512          # q-chunk / psum bank free size in f32
NCHUNK = S // CH  # 8
KT = 128          # k-tile size
VSLOT = 160       # vnat slot stride (elements); h0 V at +0, ones 64; h1 at +80

_CACHE = {}


def _build_nc(debug_taps=False):
    import concourse.bass as bass
    import concourse.bacc as bacc
    import concourse.mybir as mybir
    import concourse.tile as tile

    f32 = mybir.dt.float32
    bf16 = mybir.dt.bfloat16
    EXP = mybir.ActivationFunctionType.Exp

    nc = bacc.Bacc("TRN2", target_bir_lowering=False, debug=False)

    xt_d = nc.dram_tensor("xt", [F, S], bf16, kind="ExternalInput")
    # weights pre-packed on host into lhsT tile layout (128, 8*128)
    wq_d = nc.dram_tensor("wq", [128, F], bf16, kind="ExternalInput")
    wk_d = nc.dram_tensor("wk", [128, F], bf16, kind="ExternalInput")
    wv_d = nc.dram_tensor("wv", [128, F], bf16, kind="ExternalInput")
    cos_d = nc.dram_tensor("cos", [128, S], bf16, kind="ExternalInput")
    ss_d = nc.dram_tensor("ss", [128, S], bf16, kind="ExternalInput")
    mask_d = nc.dram_tensor("mask", [128, 128], bf16, kind="ExternalInput")
    ident_d = nc.dram_tensor("ident", [128, 128], bf16, kind="ExternalInput")
    out_d = nc.dram_tensor("out", [130, S], f32, kind="ExternalOutput")
    if debug_taps:
        dbg_qt_d = nc.dram_tensor("dbg_qt", [128, S], bf16, kind="ExternalOutput")
        dbg_kt_d = nc.dram_tensor("dbg_kt", [128, S], bf16, kind="ExternalOutput")
        dbg_vn_d = nc.dram_tensor("dbg_vn", [128, NCHUNK * 4 * VSLOT], bf16,
                                  kind="ExternalOutput")

    with tile.TileContext(nc) as tc:
        with (
            tc.tile_pool(name="const", bufs=1) as cpool,
            tc.tile_pool(name="persist", bufs=1) as perpool,
            tc.tile_pool(name="xt", bufs=32) as xpool,
            tc.tile_pool(name="rope", bufs=6) as rpool,
            tc.tile_pool(name="vbf", bufs=2) as vbfpool,
            tc.tile_pool(name="p", bufs=6) as ppool,
            tc.tile_pool(name="ob", bufs=4) as obpool,
            tc.tile_pool(name="s1ps", bufs=2, space="PSUM") as s1pool,
            tc.tile_pool(name="sps", bufs=2, space="PSUM") as spool,
            tc.tile_pool(name="ops", bufs=2, space="PSUM") as opool,
        ):
            # ---- constants (ordered so A(0) unblocks fastest) ----
            wq_sb = cpool.tile([128, F], bf16, tag="wq")
            wk_sb = cpool.tile([128, F], bf16, tag="wk")
            wv_sb = cpool.tile([128, F], bf16, tag="wv")
            cos_sb = cpool.tile([128, S], bf16, tag="cos")
            ss_sb = cpool.tile([128, S], bf16, tag="ss")
            mask_sb = cpool.tile([128, 128], bf16, tag="mask")
            ident_sb = cpool.tile([128, 128], bf16, tag="ident")

            xts = {}  # (c, ft) -> tile

            def load_x(c):
                for ft in range(8):
                    t = xpool.tile([128, CH], bf16, tag="xt", name=f"xt{c}_{ft}")
                    nc.sync.dma_start(
                        t[:], xt_d[ft * 128:(ft + 1) * 128, c * CH:(c + 1) * CH])
                    xts[(c, ft)] = t

            # contiguous weight loads first, then x(0) and halved cos/ss
            # so every queue pulls its share early
            nc.sync.dma_start(wq_sb[:], wq_d[:])
            nc.sync.dma_start(wk_sb[:], wk_d[:])
            nc.sync.dma_start(wv_sb[:], wv_d[:])
            load_x(0)
            ht = S // 2
            for j in range(2):
                nc.sync.dma_start(cos_sb[:, j * ht:(j + 1) * ht],
                                  cos_d[:, j * ht:(j + 1) * ht])
            for j in range(2):
                nc.sync.dma_start(ss_sb[:, j * ht:(j + 1) * ht],
                                  ss_d[:, j * ht:(j + 1) * ht])
            nc.sync.dma_start(mask_sb[:], mask_d[:])
            nc.sync.dma_start(ident_sb[:], ident_d[:])
            load_x(1)
            load_x(2)
            load_x(3)

            # persistent per-chunk tiles
            qT = [perpool.tile([128, CH], bf16, tag=f"qT{c}", name=f"qT{c}")
                  for c in range(NCHUNK)]
            kT = [perpool.tile([128, CH], bf16, tag=f"kT{c}", name=f"kT{c}")
                  for c in range(NCHUNK)]
            vnat = [perpool.tile([128, 4 * VSLOT], bf16, tag=f"vn{c}", name=f"vn{c}")
                    for c in range(NCHUNK)]
            for c in range(NCHUNK):
                v3 = vnat[c].rearrange("p (t e) -> p t e", e=VSLOT)
                nc.vector.memset(v3[:, :, 64:65], 1.0)
                nc.vector.memset(v3[:, :, 144:145], 1.0)

            def rope(ps, dest):
                # dest = ps*cos + swap32(ps)*ss   (dest bf16, ps f32 PSUM)
                # one PSUM->bf16 copy, then all-bf16 SBUF ops at 2x DVE rate;
                # the copy is also psQ/psK's only reader, so the next stage-A
                # accumulation can reuse the PSUM bank without waiting on
                # cos/ss-gated multiplies.
                c = dest_chunk[id(dest)]
                sl = slice(c * CH, (c + 1) * CH)
                raw = rpool.tile([128, CH], bf16, tag="raw")
                sw = rpool.tile([128, CH], bf16, tag="sw")
                t1 = rpool.tile([128, CH], bf16, tag="t1")
                nc.vector.tensor_copy(raw[:], ps[:])
                nc.vector.tensor_mul(t1[:], raw[:], cos_sb[:, sl])
                for b in range(4):
                    src = slice((b ^ 1) * 32, ((b ^ 1) + 1) * 32)
                    dst = slice(b * 32, (b + 1) * 32)
                    nc.vector.tensor_copy(sw[dst, :], raw[src, :])
                nc.vector.tensor_mul(sw[:], sw[:], ss_sb[:, sl])
                nc.vector.tensor_add(dest[:], t1[:], sw[:])

            dest_chunk = {}

            # ---- stage A emitters (projection for chunk c) ----
            def emit_A_q(c):
                ps = s1pool.tile([128, CH], f32, tag="s1", name=f"psQ{c}")
                for ft in range(8):
                    nc.tensor.matmul(
                        ps[:], lhsT=wq_sb[:, ft * 128:(ft + 1) * 128],
                        rhs=xts[(c, ft)][:], start=(ft == 0), stop=(ft == 7))
                dest_chunk[id(qT[c])] = c
                rope(ps, qT[c])

            def emit_A_k(c):
                ps = s1pool.tile([128, CH], f32, tag="s1", name=f"psK{c}")
                for ft in range(8):
                    nc.tensor.matmul(
                        ps[:], lhsT=wk_sb[:, ft * 128:(ft + 1) * 128],
                        rhs=xts[(c, ft)][:], start=(ft == 0), stop=(ft == 7))
                dest_chunk[id(kT[c])] = c
                rope(ps, kT[c])

            def emit_A_v(c):
                ps = s1pool.tile([128, CH], f32, tag="s1", name=f"psV{c}")
                for ft in range(8):
                    nc.tensor.matmul(
                        ps[:], lhsT=wv_sb[:, ft * 128:(ft + 1) * 128],
                        rhs=xts[(c, ft)][:], start=(ft == 0), stop=(ft == 7))
                vbf = vbfpool.tile([128, CH], bf16, tag="vbf", name=f"vbf{c}")
                nc.vector.tensor_copy(vbf[:], ps[:])
                return vbf

            def emit_A_vt(c, vbf):
                # transpose V to natural (s, d) layout through the PE
                psT = s1pool.tile([128, CH], bf16, tag="s1", name=f"psT{c}")
                for j in range(4):
                    nc.tensor.transpose(
                        psT[:, j * 128:(j + 1) * 128],
                        vbf[:, j * 128:(j + 1) * 128],
                        ident_sb[:],
                    )
                p3 = psT.rearrange("p (t e) -> p t e", e=128)
                v3 = vnat[c].rearrange("p (t e) -> p t e", e=VSLOT)
                nc.vector.tensor_copy(v3[:, :, 0:64], p3[:, :, 0:64])
                nc.vector.tensor_copy(v3[:, :, 80:144], p3[:, :, 64:128])

            # ---- fused schedule ----
            def emit_B(c, a_steps):
                """Attention for q-chunk c, with stage-A thunks for chunk c+1
                spread between groups. PV runs one group behind scores."""
                nkt = 4 * c + 4
                oT = [opool.tile([65, CH], f32, tag="oT", name=f"oT{c}_{h}")
                      for h in range(2)]
                # insertion points for a_steps across the group list
                ins_at = {}
                if a_steps:
                    for i, step in enumerate(a_steps):
                        g = min(nkt - 1, 1 + (i * max(1, nkt - 1)) // len(a_steps))
                        ins_at.setdefault(g, []).append(step)
                pending = None  # (kt, pt, n, off)

                def emit_pv(kt, pt, n, off):
                    for h in range(2):
                        nc.tensor.matmul(
                            oT[h][0:65, off:CH],
                            lhsT=vnat[kt // 4][:, (kt % 4) * VSLOT + 80 * h:
                                              (kt % 4) * VSLOT + 80 * h + 65],
                            rhs=pt[:, h * CH:h * CH + n],
                            start=(kt == 0), stop=(kt == nkt - 1))

                for kt in range(nkt):
                    m = kt - 4 * c
                    off = 128 * max(m, 0)
                    n = CH - off
                    sps = spool.tile([128, 2 * CH], f32, tag="sps",
                                     name=f"sps{c}_{kt}")
                    for h in range(2):
                        nc.tensor.matmul(
                            sps[:, h * CH:h * CH + n],
                            lhsT=kT[kt // 4][64 * h:64 * h + 64,
                                             (kt % 4) * 128:(kt % 4 + 1) * 128],
                            rhs=qT[c][64 * h:64 * h + 64, off:CH],
                            start=True, stop=True)
                    pt = ppool.tile([128, 2 * CH], bf16, tag="pt",
                                    name=f"pt{c}_{kt}")
                    s2 = sps.rearrange("p (g e) -> p g e", g=2)
                    p2 = pt.rearrange("p (g e) -> p g e", g=2)
                    nc.scalar.activation(
                        p2[:, :, 0:n], s2[:, :, 0:n], EXP,
                        scale=float(HD) ** -0.5)
                    if m >= 0:
                        for h in range(2):
                            nc.vector.tensor_mul(
                                pt[:, h * CH:h * CH + 128],
                                pt[:, h * CH:h * CH + 128],
                                mask_sb[:])
                    if pending is not None:
                        emit_pv(*pending)
                    pending = (kt, pt, n, off)
                    for step in ins_at.get(kt, []):
                        step()
                if pending is not None:
                    emit_pv(*pending)
                for h in range(2):
                    ob = obpool.tile([65, CH], f32, tag="ob", name=f"ob{c}_{h}")
                    nc.vector.tensor_copy(ob[:], oT[h][:])
                    nc.sync.dma_start(
                        out_d[65 * h:65 * h + 65, c * CH:(c + 1) * CH], ob[:])

            # A(0..2) front-loaded so exp work accumulates early (small B
            # phases would otherwise starve the scalar engine); then B(c)
            # runs with A(c+3) spread between its groups.
            def emit_A(c):
                emit_A_q(c)
                emit_A_k(c)
                vbf = emit_A_v(c)
                emit_A_vt(c, vbf)

            emit_A(0)
            emit_A(1)
            emit_A(2)
            for c in range(NCHUNK):
                ca = c + 3
                if ca < NCHUNK:
                    steps = []
                    if ca + 1 < NCHUNK:
                        steps.append(lambda cc=ca: load_x(cc + 1))
                    holder = {}
                    steps += [
                        lambda cc=ca: emit_A_q(cc),
                        lambda cc=ca: emit_A_k(cc),
                        lambda cc=ca, hh=holder: hh.__setitem__(
                            "vbf", emit_A_v(cc)),
                        lambda cc=ca, hh=holder: emit_A_vt(cc, hh["vbf"]),
                    ]
                else:
                    steps = []
                emit_B(c, steps)

            if debug_taps:
                for c in range(NCHUNK):
                    sl = slice(c * CH, (c + 1) * CH)
                    nc.sync.dma_start(dbg_qt_d[:, sl], qT[c][:])
                    nc.sync.dma_start(dbg_kt_d[:, sl], kT[c][:])
                    nc.sync.dma_start(
                        dbg_vn_d[:, c * 4 * VSLOT:(c + 1) * 4 * VSLOT],
                        vnat[c][:])

    nc.compile()
    return nc


def _host_inputs(x, W_kqv, b_kqv):
    """Per-core input maps. Host work is layout/constants only."""
    f32 = np.float32
    bf16 = ml_dtypes.bfloat16
    xT = np.ascontiguousarray(x.T).astype(bf16)

    ts = (10000.0 ** (2.0 * np.arange(32) / HD)).astype(np.float64)
    pos = np.arange(S, dtype=np.float64)
    ang = pos[None, :] / ts[:, None]            # (32, S)
    cos32 = np.cos(ang)
    sin32 = np.sin(ang)
    cos128 = np.tile(cos32, (4, 1)).astype(bf16)
    sgn = np.where((np.arange(128) % 64) < 32, -1.0, 1.0)[:, None]
    ss128 = (np.tile(sin32, (4, 1)) * sgn).astype(bf16)

    ident = np.eye(128, dtype=bf16)
    ki = np.arange(128)[:, None]
    qi = np.arange(128)[None, :]
    mask = (ki <= qi).astype(f32).astype(bf16)   # (128, 128) tril pass p<=j

    def pack_w(w):
        # (1024, 128) -> lhsT tiles (128 f, 8 tiles, 128 c) as (128, 1024)
        return np.ascontiguousarray(
            w.reshape(8, 128, 128).transpose(1, 0, 2).reshape(128, F)
        ).astype(bf16)

    in_maps = []
    for i in range(NCORES):
        in_maps.append({
            "xt": xT,
            "wq": pack_w(W_kqv[:, 128 * i:128 * i + 128]),
            "wk": pack_w(W_kqv[:, F + 128 * i:F + 128 * i + 128]),
            "wv": pack_w(W_kqv[:, 2 * F + 128 * i:2 * F + 128 * i + 128]),
            "cos": cos128,
            "ss": ss128,
            "mask": mask,
            "ident": ident,
        })
    return in_maps


def _assemble(results):
    y = np.empty((S, F), np.float32)
    for i in range(NCORES):
        o = results[i]["out"]  # (130, S)
        for h in range(2):
            num = o[65 * h:65 * h + 64, :]
            den = o[65 * h + 64:65 * h + 65, :]
            hg = 2 * i + h
            y[:, HD * hg:HD * hg + HD] = (num / den).T
    return y


def kernel(x, W_kqv, b_kqv):
    from concourse import bass_utils

    if "nc" not in _CACHE:
        _CACHE["nc"] = _build_nc()
    nc = _CACHE["nc"]
    in_maps = _host_inputs(np.asarray(x), np.asarray(W_kqv), np.asarray(b_kqv))
    res = bass_utils.run_bass_kernel_spmd(nc, in_maps, core_ids=list(range(NCORES)))
    return _assemble(res.results)
